# revision 16
# baseline (speedup 1.0000x reference)
"""Chunked GLA forward (nn_Gen2SingleInputReadout) as a Trainium2 Bass/Tile kernel.

Math (per batch element b, per chunk of C=128 timesteps):
    v = x @ Wv^T + bv                         (T, d=512)
    k/q = x @ W^T + b                         (T, n=128)
    alpha = sigmoid(x @ Wa^T + ba)            (T, n)
    cp[t]   = cumprod(alpha) within chunk
    A[t,s]  = sum_n (q[t]*cp[t])_n * (k[s]/(cp[s]+EPS))_n ,  masked s<=t
    y[t]    = sum_{s<=t} A[t,s] v[s]

The inter-chunk state term is scaled by a full-chunk cumprod of
~sigmoid(N(0,0.45)) ~ e^-92, i.e. far below fp32 resolution; it is dropped,
making all chunks independent (as in the fp16 baseline).

Projections run as fp8e4 DoubleRow matmuls (0.5 cyc/row, K_eff=256 per
instruction, 4x the bf16 row rate).  e4m3's ~2.2% rounding would fail the
gate, so v/k/q use a 3-term decomposition computed from host-prepacked
operands (S=256 scales keep everything out of e4m3's subnormal range):

    S*(x@W) ~= x8@fp8(S*W) + fp8(16(x-x8))@fp8(S/16*W) + x8@fp8(S*W - fp8(S*W))

with the 1/S descale folded into existing evacuation/gate ops for free.
The alpha head tolerates a single term.  kt/qt/atm/v stay bf16 (1/cp spans
1e8 via the EPS clamp -- no fp8 range), so AT and the y matmuls stay bf16.
Measured numpy end-to-end error of this scheme: ~0.6% max-norm vs fp64.

Sharding: batch B=8 -> one batch element per NeuronCore (8 cores).

Schedule (per core): software pipeline over chunk pairs, stage C (attention+
output) of pair p-1 interleaved into stage A of pair p:

    PE   : AT(p-1) | K(p) 6xDR | Q(p) 6xDR | za(p+1) 2xDR |
           y(p-1) h0,h1 | V(p) 12xDR            (~2560 cyc = 2.56us cadence)
    ACT  : sigmoid(p+1), v-evac(p) h0/h1 (scale 1/S), y-evac(p-1) h0
    DVE  : 1/x(p+1), kt(p), qt(p), mask(p-1), y-evac(p-1) h1
    POOL : cumprod scans (p+1) x2, +EPS/scale (p+1)    (gpsimd, SBUF-only)
    SP   : xr8 prefetch + y out DMAs (all transfers serialize at
           ~0.36 ns/B/partition; steady need 1.7us/pair < cadence)

Startup: DMA order [wkqa 1.75KB | xr8(p0) 2KB | wv-main 2KB | xr8(p1) |
wv-corr 4KB | xr8(p2)]; PE warms on fp32 zero-matmuls during the wait.
The final pair's AT runs inside the last stage-A cycle; its two y chunks
evacuate on ACT and DVE in parallel and DMA separately to shorten the tail.

PSUM banks (8 x 2KB): za(2) kq(1) v(2) at(1) y(2).
"""

import numpy as np
import ml_dtypes

import concourse.bass as bass
import concourse.bacc as bacc
import concourse.tile as tile
import concourse.mybir as mybir
from concourse.bass_utils import run_bass_kernel_spmd
from concourse.masks import make_upper_triangular

F32 = mybir.dt.float32
F16 = mybir.dt.float16
BF16 = mybir.dt.bfloat16
F8 = mybir.dt.float8e4
AF = mybir.ActivationFunctionType
ALU = mybir.AluOpType
PM = mybir.MatmulPerfMode

T, B, I = 2048, 8, 512      # time, batch, in_dim
D, N = 512, 128             # d_value, d_key
C = 128                     # chunk
NCH = T // C                # 16 chunks
NPAIR = NCH // 2            # 8 chunk pairs
EPS = 1e-8
NCORES = 8
S = 256.0                   # fp8 weight scale
RS = 16.0                   # x-residual upscale

NP_F8 = ml_dtypes.float8_e4m3


def build_nc(zero_bias: bool):
    nc = bacc.Bacc("TRN2", target_bir_lowering=False, debug=False)

    # x8|r8 interleaved stream: [p, pair, plane(4x x8_j | 4x r8_j), t]
    xr8_r = nc.dram_tensor("xr8_r", [128, NPAIR, 8, 256], F8,
                           kind="ExternalInput")
    # k/q/a weights (stationary operands): planes (Wk1,Wk2,Rk1,Wq1,Wq2,Rq1,Wa1)
    wkqa_r = nc.dram_tensor("wkqa_r", [128, 7, 4, N], F8, kind="ExternalInput")
    # v weights (moving operands): planes (Wv1, Wv2, Rv1)
    wv_r = nc.dram_tensor("wv_r", [128, 3, 4, D], F8, kind="ExternalInput")
    biases = None
    if not zero_bias:
        biases = {
            "bv": nc.dram_tensor("bv", [1, D], F32, kind="ExternalInput"),
            "bks": nc.dram_tensor("bks", [N, 1], F32, kind="ExternalInput"),
            "bqs": nc.dram_tensor("bqs", [N, 1], F32, kind="ExternalInput"),
            "ba": nc.dram_tensor("ba", [N, 1], F32, kind="ExternalInput"),
        }
    y = nc.dram_tensor("y", [T, D], F16, kind="ExternalOutput")

    with tile.TileContext(nc) as tc:
        _emit(tc, xr8_r, wkqa_r, wv_r, biases, y)
    nc.compile()
    return nc


def _emit(tc, xr8_r, wkqa_r, wv_r, biases, y):
    nc = tc.nc
    import contextlib

    ctx = contextlib.ExitStack()
    const = ctx.enter_context(tc.tile_pool(name="const", bufs=1))
    xin = ctx.enter_context(tc.tile_pool(name="xin", bufs=4))
    work = ctx.enter_context(tc.tile_pool(name="work", bufs=3))
    gate = ctx.enter_context(tc.tile_pool(name="gate", bufs=3))
    vout = ctx.enter_context(tc.tile_pool(name="vout", bufs=4))
    yout = ctx.enter_context(tc.tile_pool(name="yout", bufs=3))
    ps_za = ctx.enter_context(tc.tile_pool(name="ps_za", bufs=2, space="PSUM"))
    ps_kq = ctx.enter_context(tc.tile_pool(name="ps_kq", bufs=1, space="PSUM"))
    ps_v = ctx.enter_context(tc.tile_pool(name="ps_v", bufs=2, space="PSUM"))
    ps_at = ctx.enter_context(tc.tile_pool(name="ps_at", bufs=1, space="PSUM"))
    ps_y = ctx.enter_context(tc.tile_pool(name="ps_y", bufs=2, space="PSUM"))

    with ctx:
        # ---- preamble: constants + ACT table preload, all in DMA dead time.
        zeros = const.tile([128, C], F32, tag="zeros", name="zeros")
        nc.gpsimd.memset(zeros[:], 0.0)
        dummy = const.tile([1, 2], F32, tag="dummy", name="dummy")
        nc.scalar.activation(dummy[:, 0:1], zeros[0:1, 0:1], AF.Sigmoid,
                             scale=1.0)
        nc.scalar.copy(dummy[:, 1:2], zeros[0:1, 0:1])
        # U2 = [U | U]: one upper-triangular C x C mask per chunk half.
        U2 = const.tile([C, 2 * C], F32, tag="umask", name="umask")
        make_upper_triangular(nc, U2[:, 0:C], val=1.0, diag=True)
        make_upper_triangular(nc, U2[:, C : 2 * C], val=1.0, diag=True)

        # ---- input DMAs, sync (SP) HWDGE queue only -- y outputs go on the
        # scalar queue so input prefetches never wait behind evac-gated
        # output issues.  Startup splits ordered by first use; every DMA's
        # completion semaphore lags its transfer by 900ns, so consumers are
        # fed a piece at a time.
        xr8 = [None] * NPAIR
        wkqa = const.tile([128, 7, 4, N], F8, tag="wkqa", name="wkqa")
        nc.sync.dma_start(wkqa[:], wkqa_r[:])                 # za + k/q wts
        xr8[0] = xin.tile([128, 8, 256], F8, tag="xr8", name="xr8_0")
        nc.sync.dma_start(xr8[0][:], xr8_r[:, 0])
        xr8[1] = xin.tile([128, 8, 256], F8, tag="xr8", name="xr8_1")
        nc.sync.dma_start(xr8[1][:, 0:4], xr8_r[:, 1, 0:4])   # x8(p1): za(1)
        wv = const.tile([128, 3, 4, D], F8, tag="wv", name="wv")
        nc.sync.dma_start(wv[:, 0], wv_r[:, 0])               # Wv1 (term 1)
        nc.sync.dma_start(wv[:, 2], wv_r[:, 2])               # Rv1 (term 3)
        nc.sync.dma_start(wv[:, 1], wv_r[:, 1])               # Wv2 (term 2)
        nc.sync.dma_start(xr8[1][:, 4:8], xr8_r[:, 1, 4:8])   # r8(p1)
        xr8[2] = xin.tile([128, 8, 256], F8, tag="xr8", name="xr8_2")
        nc.sync.dma_start(xr8[2][:], xr8_r[:, 2])
        xr8[3] = xin.tile([128, 8, 256], F8, tag="xr8", name="xr8_3")
        nc.sync.dma_start(xr8[3][:], xr8_r[:, 3])

        wk = [wkqa[:, 0], wkqa[:, 1], wkqa[:, 2]]   # t1, t2, t3
        wq = [wkqa[:, 3], wkqa[:, 4], wkqa[:, 5]]
        wa1 = wkqa[:, 6]

        bias_sb = None
        if biases is not None:
            bias_sb = {}
            for nm in ("ba", "bks", "bqs"):
                t = const.tile([N, 1], F32, tag=nm, name=nm)
                nc.scalar.dma_start(t[:], biases[nm][:])
                bias_sb[nm] = t
            bv_sb = const.tile([1, D], F32, tag="bv", name="bv")
            nc.scalar.dma_start(bv_sb[:], biases["bv"][:])
            bv_full = const.tile([C, D], F32, tag="bvfull", name="bvfull")
            nc.gpsimd.partition_broadcast(bv_full[:], bv_sb[:])
            bias_sb["bv_full"] = bv_full

        # ---- PE p-state warm-up on throwaway fp32 work during the DMA wait.
        for _ in range(7):
            warm = ps_y.tile([C, C], F32, tag="y", name="warm")
            nc.tensor.matmul(warm[:], zeros[:], zeros[:], start=True, stop=True)

        def x8_mv(p, h):
            """x8 moving operand [128, 2, 256] for K-half h of pair p."""
            return xr8[p][:, 2 * h : 2 * h + 2, :]

        def r8_mv(p, h):
            return xr8[p][:, 4 + 2 * h : 6 + 2 * h, :]

        def x8_st(p, h, cc):
            """x8 stationary [128, 2, 128] for chunk cc of pair p."""
            return xr8[p][:, 2 * h : 2 * h + 2, 128 * cc : 128 * cc + 128]

        def r8_st(p, h, cc):
            return xr8[p][:, 4 + 2 * h : 6 + 2 * h, 128 * cc : 128 * cc + 128]

        def emit_gate_head(pp):
            """za + sigmoid + cumprod + 1/(S(cp+EPS)) for pair pp.

            Emitted one cycle early (during cycle pp-1) so the chain latency
            stays off the cadence.  cp' = cp/S (scan initial=1/S) and
            invp' = 1/(S cp + S EPS) fold all fp8 descales for free.
            """
            za = ps_za.tile([N, 256], F32, tag="za", name="za")
            for h in range(2):
                nc.tensor.matmul(za[:], wa1[:, 2 * h : 2 * h + 2, :],
                                 x8_mv(pp, h), start=(h == 0), stop=(h == 1),
                                 perf_mode=PM.DoubleRow)
            alpha = work.tile([N, 256], F32, tag="alpha", name="alpha")
            ba = bias_sb["ba"][:] if bias_sb is not None else 0.0
            nc.scalar.activation(alpha[:], za[:], AF.Sigmoid, bias=ba,
                                 scale=1.0 / S)
            # W = [invp | cp]: one tile so the kt/qt descale+multiply later
            # fuses into a single 512-wide DVE op.
            w = work.tile([N, 512], F32, tag="gatew", name="gatew")
            cp = w[:, 256:512]
            invp = w[:, 0:256]
            for h in range(2):
                hh = slice(h * C, (h + 1) * C)
                nc.vector.tensor_tensor_scan(
                    cp[:, hh], alpha[:, hh], zeros[:], 1.0,
                    ALU.mult, ALU.add,
                )
            nc.gpsimd.tensor_scalar_add(invp[:], cp[:], EPS)
            nc.vector.reciprocal_approx_fast(invp[:], invp[:])
            return w

        def emit_at_mask(kt_p, qt_p):
            at = ps_at.tile([C, 2 * C], F32, tag="at", name="at")
            for h in range(2):
                hh = slice(h * C, (h + 1) * C)
                nc.tensor.matmul(at[:, hh], kt_p[:, hh], qt_p[:, hh],
                                 start=True, stop=True)
            atm = gate.tile([C, 2 * C], BF16, tag="atm", name="atm")
            nc.vector.tensor_mul(atm[:], at[:], U2[:])
            return atm

        head = {0: emit_gate_head(0)}
        hist = {}  # pair -> (v_sb[2], atm)
        for p in range(NPAIR + 1):
            stage_a = p < NPAIR
            cpair = p - 1
            stage_c = cpair >= 0
            prev = hist.pop(cpair, None)
            last = p == NPAIR

            if stage_a and p + 4 < NPAIR:
                xr8[p + 4] = xin.tile([128, 8, 256], F8, tag="xr8",
                                      name=f"xr8_{p + 4}")
                nc.sync.dma_start(xr8[p + 4][:], xr8_r[:, p + 4])

            atm = None
            if stage_c:
                v_p, atm = prev

            kt = qt = None
            if stage_a:
                # K and Q: 6 DoubleRow matmuls each (3 terms x 2 K-halves),
                # grouped by moving operand.
                kq = ps_kq.tile([N, 512], F32, tag="kq", name="kq")
                for col, w3 in ((slice(0, 256), wk), (slice(256, 512), wq)):
                    mms = []
                    for h in range(2):
                        mms.append((w3[0][:, 2 * h : 2 * h + 2, :], x8_mv(p, h)))
                        mms.append((w3[2][:, 2 * h : 2 * h + 2, :], x8_mv(p, h)))
                        mms.append((w3[1][:, 2 * h : 2 * h + 2, :], r8_mv(p, h)))
                    for j, (lhsT, rhs) in enumerate(mms):
                        nc.tensor.matmul(kq[:, col], lhsT, rhs,
                                         start=(j == 0), stop=(j == 5),
                                         perf_mode=PM.DoubleRow)

                w = head.pop(p)
                ktqt = gate.tile([N, 512], BF16, tag="ktqt", name="ktqt")
                kt = ktqt[:, 0:256]
                qt = ktqt[:, 256:512]
                if bias_sb is None:
                    # fused: [kt|qt] = (S*[k|q] * 1/S) * [invp|cp]
                    nc.vector.scalar_tensor_tensor(
                        ktqt[:], kq[:], 1.0 / S, w[:], ALU.mult, ALU.mult)
                else:
                    kqs = work.tile([N, 512], F32, tag="kqs", name="kqs")
                    nc.vector.tensor_scalar(kqs[:], kq[:], 1.0 / S, 0.0,
                                            ALU.mult, ALU.add)
                    nc.vector.scalar_tensor_tensor(
                        kt, kqs[:, 0:256], bias_sb["bks"][:], w[:, 0:256],
                        ALU.add, ALU.mult)
                    nc.vector.scalar_tensor_tensor(
                        qt, kqs[:, 256:512], bias_sb["bqs"][:], w[:, 256:512],
                        ALU.add, ALU.mult)

                if p + 1 < NPAIR:
                    head[p + 1] = emit_gate_head(p + 1)

            def emit_y_block():
                ys = yout.tile([C, 2, D], F16, tag="ys", name="ys")
                if not last:
                    for h in range(2):
                        yp = ps_y.tile([C, D], F32, tag="y", name=f"yp{h}")
                        nc.tensor.matmul(yp[:], atm[:, h * C : (h + 1) * C],
                                         v_p[h][:], start=True, stop=True)
                        # h0 evac on ACT, h1 on DVE (engine balance); in the
                        # final stage-A cycle keep DVE free for ktqt/atm(7)
                        if h == 0 or cpair == NPAIR - 2:
                            nc.scalar.copy(ys[:, h, :], yp[:])
                        else:
                            nc.vector.tensor_copy(ys[:, 1, :], yp[:])
                        cidx = cpair * 2 + h
                        nc.sync.dma_start(y[cidx * C : (cidx + 1) * C, :],
                                          ys[:, h, :])
                    return
                # Final pair: chunk h0 full-width; chunk h1 as two half-d
                # matmuls with parallel ACT/DVE evacs and a single DMA, both
                # DMAs on the idle sync queue (lower DGE delay).
                yp0 = ps_y.tile([C, D], F32, tag="y", name="yp0")
                nc.tensor.matmul(yp0[:], atm[:, 0:C], v_p[0][:],
                                 start=True, stop=True)
                nc.vector.tensor_copy(ys[:, 0, :], yp0[:])
                c0 = cpair * 2
                nc.sync.dma_start(y[c0 * C : (c0 + 1) * C, :], ys[:, 0, :])
                yp1 = ps_y.tile([C, D], F32, tag="y", name="yp1")
                nc.tensor.matmul(yp1[:], atm[:, C : 2 * C], v_p[1][:],
                                 start=True, stop=True)
                nc.scalar.copy(ys[:, 1, 0:256], yp1[:, 0:256])
                nc.vector.tensor_copy(ys[:, 1, 256:512], yp1[:, 256:512])
                c1 = cpair * 2 + 1
                nc.sync.dma_start(y[c1 * C : (c1 + 1) * C, :], ys[:, 1, :])

            def emit_v_chunk(cc):
                    vp = ps_v.tile([C, D], F32, tag="v", name="v")
                    mms = []
                    for h in range(2):
                        mms.append((x8_st(p, h, cc), wv[:, 0, 2 * h : 2 * h + 2, :]))
                    for h in range(2):
                        mms.append((x8_st(p, h, cc), wv[:, 2, 2 * h : 2 * h + 2, :]))
                    for h in range(2):
                        mms.append((r8_st(p, h, cc), wv[:, 1, 2 * h : 2 * h + 2, :]))
                    for j, (lhsT, rhs) in enumerate(mms):
                        nc.tensor.matmul(vp[:], lhsT, rhs,
                                         start=(j == 0), stop=(j == 5),
                                         perf_mode=PM.DoubleRow)
                    vs = vout.tile([C, D], BF16, tag="vs", name="vs")
                    if bias_sb is None:
                        nc.scalar.activation(vs[:], vp[:], AF.Copy,
                                             scale=1.0 / S)
                    else:
                        nc.vector.scalar_tensor_tensor(
                            vs[:], vp[:], 1.0 / S, bias_sb["bv_full"][:],
                            ALU.mult, ALU.add)
                    return vs

            def emit_v_block():
                return [emit_v_chunk(0), emit_v_chunk(1)]

            if stage_c:
                emit_y_block()
            if stage_a:
                if p == NPAIR - 1:
                    v_sb = [emit_v_chunk(0)]
                    atm_pre = emit_at_mask(kt, qt)
                    v_sb.append(emit_v_chunk(1))
                else:
                    atm_pre = emit_at_mask(kt, qt)
                    v_sb = emit_v_block()
                hist[p] = (v_sb, atm_pre)


_NC_CACHE = {}


def _get_nc(zero_bias=True):
    if zero_bias not in _NC_CACHE:
        _NC_CACHE[zero_bias] = build_nc(zero_bias)
    return _NC_CACHE[zero_bias]


def _q8(a):
    return np.asarray(a, np.float32).astype(NP_F8)


def _pack_w(w, cols):
    """(cols, I) f32 -> [128, 4, cols] with i = 128j + p (fp32 values)."""
    wT = np.asarray(w, np.float32).T.reshape(4, 128, cols)
    return np.ascontiguousarray(wT.transpose(1, 0, 2))


def _w_terms(w, cols):
    """3-term fp8 set for one weight matrix: (W1, W2, R1), each [128,4,cols]."""
    wp = _pack_w(w, cols)
    w1 = _q8(S * wp)
    w2 = _q8((S / RS) * wp)
    r1 = _q8(S * wp - w1.astype(np.float32))
    return w1, w2, r1


def make_in_maps(x, Wv, bv, Wk, bk, Wq, bq, Wa, ba, zero_bias=True):
    x = np.asarray(x, np.float32)

    kv1, kv2, kr1 = _w_terms(Wk, N)
    qv1, qv2, qr1 = _w_terms(Wq, N)
    av1, _, _ = _w_terms(Wa, N)
    wkqa = np.ascontiguousarray(
        np.stack([kv1, kv2, kr1, qv1, qv2, qr1, av1], axis=1))

    vv1, vv2, vr1 = _w_terms(Wv, D)
    wv = np.ascontiguousarray(np.stack([vv1, vv2, vr1], axis=1))

    shared = {"wkqa_r": wkqa, "wv_r": wv}
    if not zero_bias:
        shared.update({
            "bv": np.asarray(bv, np.float32).reshape(1, D),
            "bks": np.asarray(bk, np.float32).reshape(N, 1),
            "bqs": np.asarray(bq, np.float32).reshape(N, 1),
            "ba": np.asarray(ba, np.float32).reshape(N, 1),
        })

    in_maps = []
    for b in range(NCORES):
        xb = x[:, b, :]                       # (T, I)
        x8 = _q8(xb)
        r8 = _q8(RS * (xb - x8.astype(np.float32)))
        # [p, pair, j, t]: val[t, 128j+p] with t = 256*pair + tau
        def pk(a):
            return a.T.reshape(4, 128, NPAIR, 256).transpose(1, 2, 0, 3)
        xr = np.concatenate([pk(x8), pk(r8)], axis=2)  # [128, NPAIR, 8, 256]
        in_maps.append({"xr8_r": np.ascontiguousarray(xr), **shared})
    return in_maps


def run(inputs, trace=False, **kw):
    zero_bias = all(
        not np.any(np.asarray(inputs[k])) for k in ("bv", "bk", "bq", "ba")
    )
    nc = _get_nc(zero_bias)
    in_maps = make_in_maps(**inputs, zero_bias=zero_bias)
    res = run_bass_kernel_spmd(nc, in_maps, core_ids=list(range(NCORES)),
                               trace=trace, **kw)
    out = np.stack(
        [res.results[b]["y"].astype(np.float32) for b in range(NCORES)], axis=1
    )
    return out, res


def kernel(x, Wv, bv, Wk, bk, Wq, bq, Wa, ba):
    out, _ = run(dict(x=x, Wv=Wv, bv=bv, Wk=Wk, bk=bk, Wq=Wq, bq=bq,
                      Wa=Wa, ba=ba))
    return out


# revision 17
# speedup vs baseline: 1.0059x; 1.0059x over previous
"""Chunked GLA forward (nn_Gen2SingleInputReadout) as a Trainium2 Bass/Tile kernel.

Math (per batch element b, per chunk of C=128 timesteps):
    v = x @ Wv^T + bv                         (T, d=512)
    k/q = x @ W^T + b                         (T, n=128)
    alpha = sigmoid(x @ Wa^T + ba)            (T, n)
    cp[t]   = cumprod(alpha) within chunk
    A[t,s]  = sum_n (q[t]*cp[t])_n * (k[s]/(cp[s]+EPS))_n ,  masked s<=t
    y[t]    = sum_{s<=t} A[t,s] v[s]

The inter-chunk state term is scaled by a full-chunk cumprod of
~sigmoid(N(0,0.45)) ~ e^-92, i.e. far below fp32 resolution; it is dropped,
making all chunks independent (as in the fp16 baseline).

Projections run as fp8e4 DoubleRow matmuls (0.5 cyc/row, K_eff=256 per
instruction, 4x the bf16 row rate).  e4m3's ~2.2% rounding would fail the
gate, so v/k/q use a 3-term decomposition computed from host-prepacked
operands (S=256 scales keep everything out of e4m3's subnormal range):

    S*(x@W) ~= x8@fp8(S*W) + fp8(16(x-x8))@fp8(S/16*W) + x8@fp8(S*W - fp8(S*W))

with the 1/S descale folded into existing evacuation/gate ops for free.
The alpha head tolerates a single term.  kt/qt/atm/v stay bf16 (1/cp spans
1e8 via the EPS clamp -- no fp8 range), so AT and the y matmuls stay bf16.
Measured numpy end-to-end error of this scheme: ~0.6% max-norm vs fp64.

Sharding: batch B=8 -> one batch element per NeuronCore (8 cores).

Schedule (per core): software pipeline over chunk pairs, stage C (attention+
output) of pair p-1 interleaved into stage A of pair p:

    PE   : AT(p-1) | K(p) 6xDR | Q(p) 6xDR | za(p+1) 2xDR |
           y(p-1) h0,h1 | V(p) 12xDR            (~2560 cyc = 2.56us cadence)
    ACT  : sigmoid(p+1), v-evac(p) h0/h1 (scale 1/S), y-evac(p-1) h0
    DVE  : 1/x(p+1), kt(p), qt(p), mask(p-1), y-evac(p-1) h1
    POOL : cumprod scans (p+1) x2, +EPS/scale (p+1)    (gpsimd, SBUF-only)
    SP   : xr8 prefetch + y out DMAs (all transfers serialize at
           ~0.36 ns/B/partition; steady need 1.7us/pair < cadence)

Startup: DMA order [wkqa 1.75KB | xr8(p0) 2KB | wv-main 2KB | xr8(p1) |
wv-corr 4KB | xr8(p2)]; PE warms on fp32 zero-matmuls during the wait.
The final pair's AT runs inside the last stage-A cycle; its two y chunks
evacuate on ACT and DVE in parallel and DMA separately to shorten the tail.

PSUM banks (8 x 2KB): za(2) kq(1) v(2) at(1) y(2).
"""

import numpy as np
import ml_dtypes

import concourse.bass as bass
import concourse.bacc as bacc
import concourse.tile as tile
import concourse.mybir as mybir
from concourse.bass_utils import run_bass_kernel_spmd
from concourse.masks import make_upper_triangular

F32 = mybir.dt.float32
F16 = mybir.dt.float16
BF16 = mybir.dt.bfloat16
F8 = mybir.dt.float8e4
AF = mybir.ActivationFunctionType
ALU = mybir.AluOpType
PM = mybir.MatmulPerfMode

T, B, I = 2048, 8, 512      # time, batch, in_dim
D, N = 512, 128             # d_value, d_key
C = 128                     # chunk
NCH = T // C                # 16 chunks
NPAIR = NCH // 2            # 8 chunk pairs
EPS = 1e-8
NCORES = 8
S = 256.0                   # fp8 weight scale
RS = 16.0                   # x-residual upscale

NP_F8 = ml_dtypes.float8_e4m3


def build_nc(zero_bias: bool):
    nc = bacc.Bacc("TRN2", target_bir_lowering=False, debug=False)

    # x8|r8 interleaved stream: [p, pair, plane(4x x8_j | 4x r8_j), t]
    xr8_r = nc.dram_tensor("xr8_r", [128, NPAIR, 8, 256], F8,
                           kind="ExternalInput")
    # k/q/a weights (stationary operands): planes (Wk1,Wk2,Rk1,Wq1,Wq2,Rq1,Wa1)
    wkqa_r = nc.dram_tensor("wkqa_r", [128, 7, 4, N], F8, kind="ExternalInput")
    # v weights (moving operands): planes (Wv1, Wv2, Rv1)
    wv_r = nc.dram_tensor("wv_r", [128, 3, 4, D], F8, kind="ExternalInput")
    biases = None
    if not zero_bias:
        biases = {
            "bv": nc.dram_tensor("bv", [1, D], F32, kind="ExternalInput"),
            "bks": nc.dram_tensor("bks", [N, 1], F32, kind="ExternalInput"),
            "bqs": nc.dram_tensor("bqs", [N, 1], F32, kind="ExternalInput"),
            "ba": nc.dram_tensor("ba", [N, 1], F32, kind="ExternalInput"),
        }
    y = nc.dram_tensor("y", [T, D], F16, kind="ExternalOutput")

    with tile.TileContext(nc) as tc:
        _emit(tc, xr8_r, wkqa_r, wv_r, biases, y)
    nc.compile()
    return nc


def _emit(tc, xr8_r, wkqa_r, wv_r, biases, y):
    nc = tc.nc
    import contextlib

    ctx = contextlib.ExitStack()
    const = ctx.enter_context(tc.tile_pool(name="const", bufs=1))
    xin = ctx.enter_context(tc.tile_pool(name="xin", bufs=4))
    work = ctx.enter_context(tc.tile_pool(name="work", bufs=3))
    gate = ctx.enter_context(tc.tile_pool(name="gate", bufs=3))
    vout = ctx.enter_context(tc.tile_pool(name="vout", bufs=4))
    yout = ctx.enter_context(tc.tile_pool(name="yout", bufs=3))
    ps_za = ctx.enter_context(tc.tile_pool(name="ps_za", bufs=2, space="PSUM"))
    ps_kq = ctx.enter_context(tc.tile_pool(name="ps_kq", bufs=1, space="PSUM"))
    ps_v = ctx.enter_context(tc.tile_pool(name="ps_v", bufs=2, space="PSUM"))
    ps_at = ctx.enter_context(tc.tile_pool(name="ps_at", bufs=1, space="PSUM"))
    ps_y = ctx.enter_context(tc.tile_pool(name="ps_y", bufs=2, space="PSUM"))

    with ctx:
        # ---- preamble: constants + ACT table preload, all in DMA dead time.
        zeros = const.tile([128, C], F32, tag="zeros", name="zeros")
        nc.gpsimd.memset(zeros[:], 0.0)
        dummy = const.tile([1, 2], F32, tag="dummy", name="dummy")
        nc.scalar.activation(dummy[:, 0:1], zeros[0:1, 0:1], AF.Sigmoid,
                             scale=1.0)
        nc.scalar.copy(dummy[:, 1:2], zeros[0:1, 0:1])
        # U2 = [U | U]: one upper-triangular C x C mask per chunk half.
        U2 = const.tile([C, 2 * C], F32, tag="umask", name="umask")
        make_upper_triangular(nc, U2[:, 0:C], val=1.0, diag=True)
        make_upper_triangular(nc, U2[:, C : 2 * C], val=1.0, diag=True)

        # ---- input DMAs, sync (SP) HWDGE queue only -- y outputs go on the
        # scalar queue so input prefetches never wait behind evac-gated
        # output issues.  Startup splits ordered by first use; every DMA's
        # completion semaphore lags its transfer by 900ns, so consumers are
        # fed a piece at a time.
        xr8 = [None] * NPAIR
        wkqa = const.tile([128, 7, 4, N], F8, tag="wkqa", name="wkqa")
        nc.sync.dma_start(wkqa[:], wkqa_r[:])                 # za + k/q wts
        xr8[0] = xin.tile([128, 8, 256], F8, tag="xr8", name="xr8_0")
        nc.sync.dma_start(xr8[0][:], xr8_r[:, 0])
        xr8[1] = xin.tile([128, 8, 256], F8, tag="xr8", name="xr8_1")
        nc.sync.dma_start(xr8[1][:, 0:4], xr8_r[:, 1, 0:4])   # x8(p1): za(1)
        wv = const.tile([128, 3, 4, D], F8, tag="wv", name="wv")
        nc.sync.dma_start(wv[:, 0], wv_r[:, 0])               # Wv1 (term 1)
        nc.sync.dma_start(wv[:, 2], wv_r[:, 2])               # Rv1 (term 3)
        nc.sync.dma_start(wv[:, 1], wv_r[:, 1])               # Wv2 (term 2)
        nc.sync.dma_start(xr8[1][:, 4:8], xr8_r[:, 1, 4:8])   # r8(p1)
        xr8[2] = xin.tile([128, 8, 256], F8, tag="xr8", name="xr8_2")
        nc.sync.dma_start(xr8[2][:], xr8_r[:, 2])
        xr8[3] = xin.tile([128, 8, 256], F8, tag="xr8", name="xr8_3")
        nc.sync.dma_start(xr8[3][:], xr8_r[:, 3])

        wk = [wkqa[:, 0], wkqa[:, 1], wkqa[:, 2]]   # t1, t2, t3
        wq = [wkqa[:, 3], wkqa[:, 4], wkqa[:, 5]]
        wa1 = wkqa[:, 6]

        bias_sb = None
        if biases is not None:
            bias_sb = {}
            for nm in ("ba", "bks", "bqs"):
                t = const.tile([N, 1], F32, tag=nm, name=nm)
                nc.scalar.dma_start(t[:], biases[nm][:])
                bias_sb[nm] = t
            bv_sb = const.tile([1, D], F32, tag="bv", name="bv")
            nc.scalar.dma_start(bv_sb[:], biases["bv"][:])
            bv_full = const.tile([C, D], F32, tag="bvfull", name="bvfull")
            nc.gpsimd.partition_broadcast(bv_full[:], bv_sb[:])
            bias_sb["bv_full"] = bv_full

        # ---- PE p-state warm-up on throwaway fp32 work during the DMA wait.
        for _ in range(7):
            warm = ps_y.tile([C, C], F32, tag="y", name="warm")
            nc.tensor.matmul(warm[:], zeros[:], zeros[:], start=True, stop=True)

        def x8_mv(p, h):
            """x8 moving operand [128, 2, 256] for K-half h of pair p."""
            return xr8[p][:, 2 * h : 2 * h + 2, :]

        def r8_mv(p, h):
            return xr8[p][:, 4 + 2 * h : 6 + 2 * h, :]

        def x8_st(p, h, cc):
            """x8 stationary [128, 2, 128] for chunk cc of pair p."""
            return xr8[p][:, 2 * h : 2 * h + 2, 128 * cc : 128 * cc + 128]

        def r8_st(p, h, cc):
            return xr8[p][:, 4 + 2 * h : 6 + 2 * h, 128 * cc : 128 * cc + 128]

        def emit_gate_head(pp):
            """za + sigmoid + cumprod + 1/(S(cp+EPS)) for pair pp.

            Emitted one cycle early (during cycle pp-1) so the chain latency
            stays off the cadence.  cp' = cp/S (scan initial=1/S) and
            invp' = 1/(S cp + S EPS) fold all fp8 descales for free.
            """
            za = ps_za.tile([N, 256], F32, tag="za", name="za")
            for h in range(2):
                nc.tensor.matmul(za[:], wa1[:, 2 * h : 2 * h + 2, :],
                                 x8_mv(pp, h), start=(h == 0), stop=(h == 1),
                                 perf_mode=PM.DoubleRow)
            alpha = work.tile([N, 256], F32, tag="alpha", name="alpha")
            ba = bias_sb["ba"][:] if bias_sb is not None else 0.0
            nc.scalar.activation(alpha[:], za[:], AF.Sigmoid, bias=ba,
                                 scale=1.0 / S)
            # W = [invp | cp]: one tile so the kt/qt descale+multiply later
            # fuses into a single 512-wide DVE op.
            w = work.tile([N, 512], F32, tag="gatew", name="gatew")
            cp = w[:, 256:512]
            invp = w[:, 0:256]
            for h in range(2):
                hh = slice(h * C, (h + 1) * C)
                nc.vector.tensor_tensor_scan(
                    cp[:, hh], alpha[:, hh], zeros[:], 1.0,
                    ALU.mult, ALU.add,
                )
            nc.gpsimd.tensor_scalar_add(invp[:], cp[:], EPS)
            nc.vector.reciprocal_approx_fast(invp[:], invp[:])
            return w

        def emit_at_mask(kt_p, qt_p):
            at = ps_at.tile([C, 2 * C], F32, tag="at", name="at")
            for h in range(2):
                hh = slice(h * C, (h + 1) * C)
                nc.tensor.matmul(at[:, hh], kt_p[:, hh], qt_p[:, hh],
                                 start=True, stop=True)
            atm = gate.tile([C, 2 * C], BF16, tag="atm", name="atm")
            nc.vector.tensor_mul(atm[:], at[:], U2[:])
            return atm

        head = {0: emit_gate_head(0)}
        hist = {}  # pair -> (v_sb[2], atm)
        for p in range(NPAIR + 1):
            stage_a = p < NPAIR
            cpair = p - 1
            stage_c = cpair >= 0
            prev = hist.pop(cpair, None)
            last = p == NPAIR

            if stage_a and p + 4 < NPAIR:
                xr8[p + 4] = xin.tile([128, 8, 256], F8, tag="xr8",
                                      name=f"xr8_{p + 4}")
                nc.sync.dma_start(xr8[p + 4][:], xr8_r[:, p + 4])

            atm = None
            if stage_c:
                v_p, atm = prev

            kt = qt = None
            if stage_a:
                # K and Q: 6 DoubleRow matmuls each (3 terms x 2 K-halves),
                # grouped by moving operand.
                kq = ps_kq.tile([N, 512], F32, tag="kq", name="kq")
                for col, w3 in ((slice(0, 256), wk), (slice(256, 512), wq)):
                    mms = []
                    for h in range(2):
                        mms.append((w3[0][:, 2 * h : 2 * h + 2, :], x8_mv(p, h)))
                        mms.append((w3[2][:, 2 * h : 2 * h + 2, :], x8_mv(p, h)))
                        mms.append((w3[1][:, 2 * h : 2 * h + 2, :], r8_mv(p, h)))
                    for j, (lhsT, rhs) in enumerate(mms):
                        nc.tensor.matmul(kq[:, col], lhsT, rhs,
                                         start=(j == 0), stop=(j == 5),
                                         perf_mode=PM.DoubleRow)

                w = head.pop(p)
                ktqt = gate.tile([N, 512], BF16, tag="ktqt", name="ktqt")
                kt = ktqt[:, 0:256]
                qt = ktqt[:, 256:512]
                if bias_sb is None:
                    # fused: [kt|qt] = (S*[k|q] * 1/S) * [invp|cp]
                    nc.vector.scalar_tensor_tensor(
                        ktqt[:], kq[:], 1.0 / S, w[:], ALU.mult, ALU.mult)
                else:
                    kqs = work.tile([N, 512], F32, tag="kqs", name="kqs")
                    nc.vector.tensor_scalar(kqs[:], kq[:], 1.0 / S, 0.0,
                                            ALU.mult, ALU.add)
                    nc.vector.scalar_tensor_tensor(
                        kt, kqs[:, 0:256], bias_sb["bks"][:], w[:, 0:256],
                        ALU.add, ALU.mult)
                    nc.vector.scalar_tensor_tensor(
                        qt, kqs[:, 256:512], bias_sb["bqs"][:], w[:, 256:512],
                        ALU.add, ALU.mult)

                if p + 1 < NPAIR:
                    head[p + 1] = emit_gate_head(p + 1)

            def emit_y_block():
                ys = yout.tile([C, 2, D], F16, tag="ys", name="ys")
                if not last:
                    for h in range(2):
                        yp = ps_y.tile([C, D], F32, tag="y", name=f"yp{h}")
                        nc.tensor.matmul(yp[:], atm[:, h * C : (h + 1) * C],
                                         v_p[h][:], start=True, stop=True)
                        # h0 evac on ACT, h1 on DVE (engine balance)
                        if h == 0:
                            nc.scalar.copy(ys[:, 0, :], yp[:])
                        else:
                            nc.vector.tensor_copy(ys[:, 1, :], yp[:])
                        cidx = cpair * 2 + h
                        nc.sync.dma_start(y[cidx * C : (cidx + 1) * C, :],
                                          ys[:, h, :])
                    return
                # Final pair: chunk h0 full-width; chunk h1 as two half-d
                # matmuls with parallel ACT/DVE evacs and a single DMA, both
                # DMAs on the idle sync queue (lower DGE delay).
                yp0 = ps_y.tile([C, D], F32, tag="y", name="yp0")
                nc.tensor.matmul(yp0[:], atm[:, 0:C], v_p[0][:],
                                 start=True, stop=True)
                nc.vector.tensor_copy(ys[:, 0, :], yp0[:])
                c0 = cpair * 2
                nc.sync.dma_start(y[c0 * C : (c0 + 1) * C, :], ys[:, 0, :])
                yp1 = ps_y.tile([C, D], F32, tag="y", name="yp1")
                nc.tensor.matmul(yp1[:], atm[:, C : 2 * C], v_p[1][:],
                                 start=True, stop=True)
                nc.scalar.copy(ys[:, 1, 0:256], yp1[:, 0:256])
                nc.vector.tensor_copy(ys[:, 1, 256:512], yp1[:, 256:512])
                c1 = cpair * 2 + 1
                nc.sync.dma_start(y[c1 * C : (c1 + 1) * C, :], ys[:, 1, :])

            def emit_v_chunk(cc):
                    vp = ps_v.tile([C, D], F32, tag="v", name="v")
                    mms = []
                    for h in range(2):
                        mms.append((x8_st(p, h, cc), wv[:, 0, 2 * h : 2 * h + 2, :]))
                    for h in range(2):
                        mms.append((x8_st(p, h, cc), wv[:, 2, 2 * h : 2 * h + 2, :]))
                    for h in range(2):
                        mms.append((r8_st(p, h, cc), wv[:, 1, 2 * h : 2 * h + 2, :]))
                    for j, (lhsT, rhs) in enumerate(mms):
                        nc.tensor.matmul(vp[:], lhsT, rhs,
                                         start=(j == 0), stop=(j == 5),
                                         perf_mode=PM.DoubleRow)
                    vs = vout.tile([C, D], BF16, tag="vs", name="vs")
                    if bias_sb is None:
                        nc.scalar.activation(vs[:], vp[:], AF.Copy,
                                             scale=1.0 / S)
                    else:
                        nc.vector.scalar_tensor_tensor(
                            vs[:], vp[:], 1.0 / S, bias_sb["bv_full"][:],
                            ALU.mult, ALU.add)
                    return vs

            def emit_v_block():
                return [emit_v_chunk(0), emit_v_chunk(1)]

            if stage_c:
                emit_y_block()
            if stage_a:
                if p == NPAIR - 1:
                    v_sb = [emit_v_chunk(0)]
                    atm_pre = emit_at_mask(kt, qt)
                    v_sb.append(emit_v_chunk(1))
                else:
                    atm_pre = emit_at_mask(kt, qt)
                    v_sb = emit_v_block()
                hist[p] = (v_sb, atm_pre)


_NC_CACHE = {}


def _get_nc(zero_bias=True):
    if zero_bias not in _NC_CACHE:
        _NC_CACHE[zero_bias] = build_nc(zero_bias)
    return _NC_CACHE[zero_bias]


def _q8(a):
    return np.asarray(a, np.float32).astype(NP_F8)


def _pack_w(w, cols):
    """(cols, I) f32 -> [128, 4, cols] with i = 128j + p (fp32 values)."""
    wT = np.asarray(w, np.float32).T.reshape(4, 128, cols)
    return np.ascontiguousarray(wT.transpose(1, 0, 2))


def _w_terms(w, cols):
    """3-term fp8 set for one weight matrix: (W1, W2, R1), each [128,4,cols]."""
    wp = _pack_w(w, cols)
    w1 = _q8(S * wp)
    w2 = _q8((S / RS) * wp)
    r1 = _q8(S * wp - w1.astype(np.float32))
    return w1, w2, r1


def make_in_maps(x, Wv, bv, Wk, bk, Wq, bq, Wa, ba, zero_bias=True):
    x = np.asarray(x, np.float32)

    kv1, kv2, kr1 = _w_terms(Wk, N)
    qv1, qv2, qr1 = _w_terms(Wq, N)
    av1, _, _ = _w_terms(Wa, N)
    wkqa = np.ascontiguousarray(
        np.stack([kv1, kv2, kr1, qv1, qv2, qr1, av1], axis=1))

    vv1, vv2, vr1 = _w_terms(Wv, D)
    wv = np.ascontiguousarray(np.stack([vv1, vv2, vr1], axis=1))

    shared = {"wkqa_r": wkqa, "wv_r": wv}
    if not zero_bias:
        shared.update({
            "bv": np.asarray(bv, np.float32).reshape(1, D),
            "bks": np.asarray(bk, np.float32).reshape(N, 1),
            "bqs": np.asarray(bq, np.float32).reshape(N, 1),
            "ba": np.asarray(ba, np.float32).reshape(N, 1),
        })

    in_maps = []
    for b in range(NCORES):
        xb = x[:, b, :]                       # (T, I)
        x8 = _q8(xb)
        r8 = _q8(RS * (xb - x8.astype(np.float32)))
        # [p, pair, j, t]: val[t, 128j+p] with t = 256*pair + tau
        def pk(a):
            return a.T.reshape(4, 128, NPAIR, 256).transpose(1, 2, 0, 3)
        xr = np.concatenate([pk(x8), pk(r8)], axis=2)  # [128, NPAIR, 8, 256]
        in_maps.append({"xr8_r": np.ascontiguousarray(xr), **shared})
    return in_maps


def run(inputs, trace=False, **kw):
    zero_bias = all(
        not np.any(np.asarray(inputs[k])) for k in ("bv", "bk", "bq", "ba")
    )
    nc = _get_nc(zero_bias)
    in_maps = make_in_maps(**inputs, zero_bias=zero_bias)
    res = run_bass_kernel_spmd(nc, in_maps, core_ids=list(range(NCORES)),
                               trace=trace, **kw)
    out = np.stack(
        [res.results[b]["y"].astype(np.float32) for b in range(NCORES)], axis=1
    )
    return out, res


def kernel(x, Wv, bv, Wk, bk, Wq, bq, Wa, ba):
    out, _ = run(dict(x=x, Wv=Wv, bv=bv, Wk=Wk, bk=bk, Wq=Wq, bq=bq,
                      Wa=Wa, ba=ba))
    return out


# revision 18
# speedup vs baseline: 1.0075x; 1.0015x over previous
"""Chunked GLA forward (nn_Gen2SingleInputReadout) as a Trainium2 Bass/Tile kernel.

Math (per batch element b, per chunk of C=128 timesteps):
    v = x @ Wv^T + bv                         (T, d=512)
    k/q = x @ W^T + b                         (T, n=128)
    alpha = sigmoid(x @ Wa^T + ba)            (T, n)
    cp[t]   = cumprod(alpha) within chunk
    A[t,s]  = sum_n (q[t]*cp[t])_n * (k[s]/(cp[s]+EPS))_n ,  masked s<=t
    y[t]    = sum_{s<=t} A[t,s] v[s]

The inter-chunk state term is scaled by a full-chunk cumprod of
~sigmoid(N(0,0.45)) ~ e^-92, i.e. far below fp32 resolution; it is dropped,
making all chunks independent (as in the fp16 baseline).

Projections run as fp8e4 DoubleRow matmuls (0.5 cyc/row, K_eff=256 per
instruction, 4x the bf16 row rate).  e4m3's ~2.2% rounding would fail the
gate, so v/k/q use a 3-term decomposition computed from host-prepacked
operands (S=256 scales keep everything out of e4m3's subnormal range):

    S*(x@W) ~= x8@fp8(S*W) + fp8(16(x-x8))@fp8(S/16*W) + x8@fp8(S*W - fp8(S*W))

with the 1/S descale folded into existing evacuation/gate ops for free.
The alpha head tolerates a single term.  kt/qt/atm/v stay bf16 (1/cp spans
1e8 via the EPS clamp -- no fp8 range), so AT and the y matmuls stay bf16.
Measured numpy end-to-end error of this scheme: ~0.6% max-norm vs fp64.

Sharding: batch B=8 -> one batch element per NeuronCore (8 cores).

Schedule (per core): software pipeline over chunk pairs, stage C (attention+
output) of pair p-1 interleaved into stage A of pair p:

    PE   : AT(p-1) | K(p) 6xDR | Q(p) 6xDR | za(p+1) 2xDR |
           y(p-1) h0,h1 | V(p) 12xDR            (~2560 cyc = 2.56us cadence)
    ACT  : sigmoid(p+1), v-evac(p) h0/h1 (scale 1/S), y-evac(p-1) h0
    DVE  : 1/x(p+1), kt(p), qt(p), mask(p-1), y-evac(p-1) h1
    POOL : cumprod scans (p+1) x2, +EPS/scale (p+1)    (gpsimd, SBUF-only)
    SP   : xr8 prefetch + y out DMAs (all transfers serialize at
           ~0.36 ns/B/partition; steady need 1.7us/pair < cadence)

Startup: DMA order [wkqa 1.75KB | xr8(p0) 2KB | wv-main 2KB | xr8(p1) |
wv-corr 4KB | xr8(p2)]; PE warms on fp32 zero-matmuls during the wait.
The final pair's AT runs inside the last stage-A cycle; its two y chunks
evacuate on ACT and DVE in parallel and DMA separately to shorten the tail.

PSUM banks (8 x 2KB): za(2) kq(1) v(2) at(1) y(2).
"""

import numpy as np
import ml_dtypes

import concourse.bass as bass
import concourse.bacc as bacc
import concourse.tile as tile
import concourse.mybir as mybir
from concourse.bass_utils import run_bass_kernel_spmd
from concourse.masks import make_upper_triangular

F32 = mybir.dt.float32
F16 = mybir.dt.float16
BF16 = mybir.dt.bfloat16
F8 = mybir.dt.float8e4
AF = mybir.ActivationFunctionType
ALU = mybir.AluOpType
PM = mybir.MatmulPerfMode

T, B, I = 2048, 8, 512      # time, batch, in_dim
D, N = 512, 128             # d_value, d_key
C = 128                     # chunk
NCH = T // C                # 16 chunks
NPAIR = NCH // 2            # 8 chunk pairs
EPS = 1e-8
NCORES = 8
S = 256.0                   # fp8 weight scale
RS = 16.0                   # x-residual upscale

NP_F8 = ml_dtypes.float8_e4m3


def build_nc(zero_bias: bool):
    nc = bacc.Bacc("TRN2", target_bir_lowering=False, debug=False)

    # x8|r8 interleaved stream: [p, pair, plane(4x x8_j | 4x r8_j), t]
    xr8_r = nc.dram_tensor("xr8_r", [128, NPAIR, 8, 256], F8,
                           kind="ExternalInput")
    # k/q/a weights (stationary operands): planes (Wk1,Wk2,Rk1,Wq1,Wq2,Rq1,Wa1)
    wkqa_r = nc.dram_tensor("wkqa_r", [128, 7, 4, N], F8, kind="ExternalInput")
    # v weights (moving operands): planes (Wv1, Wv2, Rv1)
    wv_r = nc.dram_tensor("wv_r", [128, 3, 4, D], F8, kind="ExternalInput")
    biases = None
    if not zero_bias:
        biases = {
            "bv": nc.dram_tensor("bv", [1, D], F32, kind="ExternalInput"),
            "bks": nc.dram_tensor("bks", [N, 1], F32, kind="ExternalInput"),
            "bqs": nc.dram_tensor("bqs", [N, 1], F32, kind="ExternalInput"),
            "ba": nc.dram_tensor("ba", [N, 1], F32, kind="ExternalInput"),
        }
    y = nc.dram_tensor("y", [T, D], F16, kind="ExternalOutput")

    with tile.TileContext(nc) as tc:
        _emit(tc, xr8_r, wkqa_r, wv_r, biases, y)
    nc.compile()
    return nc


def _emit(tc, xr8_r, wkqa_r, wv_r, biases, y):
    nc = tc.nc
    import contextlib

    ctx = contextlib.ExitStack()
    const = ctx.enter_context(tc.tile_pool(name="const", bufs=1))
    xin = ctx.enter_context(tc.tile_pool(name="xin", bufs=4))
    work = ctx.enter_context(tc.tile_pool(name="work", bufs=3))
    gate = ctx.enter_context(tc.tile_pool(name="gate", bufs=3))
    vout = ctx.enter_context(tc.tile_pool(name="vout", bufs=4))
    yout = ctx.enter_context(tc.tile_pool(name="yout", bufs=3))
    ps_za = ctx.enter_context(tc.tile_pool(name="ps_za", bufs=2, space="PSUM"))
    ps_kq = ctx.enter_context(tc.tile_pool(name="ps_kq", bufs=1, space="PSUM"))
    ps_v = ctx.enter_context(tc.tile_pool(name="ps_v", bufs=2, space="PSUM"))
    ps_at = ctx.enter_context(tc.tile_pool(name="ps_at", bufs=1, space="PSUM"))
    ps_y = ctx.enter_context(tc.tile_pool(name="ps_y", bufs=2, space="PSUM"))

    with ctx:
        # ---- preamble: constants + ACT table preload, all in DMA dead time.
        zeros = const.tile([128, C], F32, tag="zeros", name="zeros")
        nc.gpsimd.memset(zeros[:], 0.0)
        dummy = const.tile([1, 2], F32, tag="dummy", name="dummy")
        nc.scalar.activation(dummy[:, 0:1], zeros[0:1, 0:1], AF.Sigmoid,
                             scale=1.0)
        nc.scalar.copy(dummy[:, 1:2], zeros[0:1, 0:1])
        # U2 = [U | U]: one upper-triangular C x C mask per chunk half.
        U2 = const.tile([C, 2 * C], F32, tag="umask", name="umask")
        make_upper_triangular(nc, U2[:, 0:C], val=1.0, diag=True)
        make_upper_triangular(nc, U2[:, C : 2 * C], val=1.0, diag=True)

        # ---- input DMAs, sync (SP) HWDGE queue only -- y outputs go on the
        # scalar queue so input prefetches never wait behind evac-gated
        # output issues.  Startup splits ordered by first use; every DMA's
        # completion semaphore lags its transfer by 900ns, so consumers are
        # fed a piece at a time.
        xr8 = [None] * NPAIR
        wkqa = const.tile([128, 7, 4, N], F8, tag="wkqa", name="wkqa")
        nc.sync.dma_start(wkqa[:], wkqa_r[:])                 # za + k/q wts
        xr8[0] = xin.tile([128, 8, 256], F8, tag="xr8", name="xr8_0")
        nc.sync.dma_start(xr8[0][:], xr8_r[:, 0])
        xr8[1] = xin.tile([128, 8, 256], F8, tag="xr8", name="xr8_1")
        nc.sync.dma_start(xr8[1][:, 0:4], xr8_r[:, 1, 0:4])   # x8(p1): za(1)
        wv = const.tile([128, 3, 4, D], F8, tag="wv", name="wv")
        nc.sync.dma_start(wv[:, 0], wv_r[:, 0])               # Wv1 (term 1)
        nc.sync.dma_start(wv[:, 2], wv_r[:, 2])               # Rv1 (term 3)
        nc.sync.dma_start(wv[:, 1], wv_r[:, 1])               # Wv2 (term 2)
        nc.sync.dma_start(xr8[1][:, 4:8], xr8_r[:, 1, 4:8])   # r8(p1)
        xr8[2] = xin.tile([128, 8, 256], F8, tag="xr8", name="xr8_2")
        nc.sync.dma_start(xr8[2][:], xr8_r[:, 2])
        xr8[3] = xin.tile([128, 8, 256], F8, tag="xr8", name="xr8_3")
        nc.sync.dma_start(xr8[3][:], xr8_r[:, 3])

        wk = [wkqa[:, 0], wkqa[:, 1], wkqa[:, 2]]   # t1, t2, t3
        wq = [wkqa[:, 3], wkqa[:, 4], wkqa[:, 5]]
        wa1 = wkqa[:, 6]

        bias_sb = None
        if biases is not None:
            bias_sb = {}
            for nm in ("ba", "bks", "bqs"):
                t = const.tile([N, 1], F32, tag=nm, name=nm)
                nc.scalar.dma_start(t[:], biases[nm][:])
                bias_sb[nm] = t
            bv_sb = const.tile([1, D], F32, tag="bv", name="bv")
            nc.scalar.dma_start(bv_sb[:], biases["bv"][:])
            bv_full = const.tile([C, D], F32, tag="bvfull", name="bvfull")
            nc.gpsimd.partition_broadcast(bv_full[:], bv_sb[:])
            bias_sb["bv_full"] = bv_full

        # ---- PE p-state warm-up on throwaway fp32 work during the DMA wait.
        for _ in range(7):
            warm = ps_y.tile([C, C], F32, tag="y", name="warm")
            nc.tensor.matmul(warm[:], zeros[:], zeros[:], start=True, stop=True)

        def x8_mv(p, h):
            """x8 moving operand [128, 2, 256] for K-half h of pair p."""
            return xr8[p][:, 2 * h : 2 * h + 2, :]

        def r8_mv(p, h):
            return xr8[p][:, 4 + 2 * h : 6 + 2 * h, :]

        def x8_st(p, h, cc):
            """x8 stationary [128, 2, 128] for chunk cc of pair p."""
            return xr8[p][:, 2 * h : 2 * h + 2, 128 * cc : 128 * cc + 128]

        def r8_st(p, h, cc):
            return xr8[p][:, 4 + 2 * h : 6 + 2 * h, 128 * cc : 128 * cc + 128]

        def emit_gate_head(pp):
            """za + sigmoid + cumprod + 1/(S(cp+EPS)) for pair pp.

            Emitted one cycle early (during cycle pp-1) so the chain latency
            stays off the cadence.  cp' = cp/S (scan initial=1/S) and
            invp' = 1/(S cp + S EPS) fold all fp8 descales for free.
            """
            za = ps_za.tile([N, 256], F32, tag="za", name="za")
            for h in range(2):
                nc.tensor.matmul(za[:], wa1[:, 2 * h : 2 * h + 2, :],
                                 x8_mv(pp, h), start=(h == 0), stop=(h == 1),
                                 perf_mode=PM.DoubleRow)
            alpha = work.tile([N, 256], F32, tag="alpha", name="alpha")
            ba = bias_sb["ba"][:] if bias_sb is not None else 0.0
            nc.scalar.activation(alpha[:], za[:], AF.Sigmoid, bias=ba,
                                 scale=1.0 / S)
            # W = [invp | cp]: one tile so the kt/qt descale+multiply later
            # fuses into a single 512-wide DVE op.
            w = work.tile([N, 512], F32, tag="gatew", name="gatew")
            cp = w[:, 256:512]
            invp = w[:, 0:256]
            for h in range(2):
                hh = slice(h * C, (h + 1) * C)
                nc.vector.tensor_tensor_scan(
                    cp[:, hh], alpha[:, hh], zeros[:], 1.0,
                    ALU.mult, ALU.add,
                )
            nc.gpsimd.tensor_scalar_add(invp[:], cp[:], EPS)
            nc.vector.reciprocal_approx_fast(invp[:], invp[:])
            return w

        def emit_at_mask(kt_p, qt_p):
            at = ps_at.tile([C, 2 * C], F32, tag="at", name="at")
            for h in range(2):
                hh = slice(h * C, (h + 1) * C)
                nc.tensor.matmul(at[:, hh], kt_p[:, hh], qt_p[:, hh],
                                 start=True, stop=True)
            atm = gate.tile([C, 2 * C], BF16, tag="atm", name="atm")
            nc.vector.tensor_mul(atm[:], at[:], U2[:])
            return atm

        head = {0: emit_gate_head(0)}
        hist = {}  # pair -> (v_sb[2], atm)
        for p in range(NPAIR + 1):
            stage_a = p < NPAIR
            cpair = p - 1
            stage_c = cpair >= 0
            prev = hist.pop(cpair, None)
            last = p == NPAIR

            if stage_a and p + 4 < NPAIR:
                xr8[p + 4] = xin.tile([128, 8, 256], F8, tag="xr8",
                                      name=f"xr8_{p + 4}")
                nc.sync.dma_start(xr8[p + 4][:], xr8_r[:, p + 4])

            atm = None
            if stage_c:
                v_p, atm = prev

            kt = qt = None
            if stage_a:
                # K and Q: 6 DoubleRow matmuls each (3 terms x 2 K-halves),
                # grouped by moving operand.
                kq = ps_kq.tile([N, 512], F32, tag="kq", name="kq")
                for col, w3 in ((slice(0, 256), wk), (slice(256, 512), wq)):
                    mms = []
                    for h in range(2):
                        mms.append((w3[0][:, 2 * h : 2 * h + 2, :], x8_mv(p, h)))
                        mms.append((w3[2][:, 2 * h : 2 * h + 2, :], x8_mv(p, h)))
                        mms.append((w3[1][:, 2 * h : 2 * h + 2, :], r8_mv(p, h)))
                    for j, (lhsT, rhs) in enumerate(mms):
                        nc.tensor.matmul(kq[:, col], lhsT, rhs,
                                         start=(j == 0), stop=(j == 5),
                                         perf_mode=PM.DoubleRow)

                w = head.pop(p)
                ktqt = gate.tile([N, 512], BF16, tag="ktqt", name="ktqt")
                kt = ktqt[:, 0:256]
                qt = ktqt[:, 256:512]
                if bias_sb is None:
                    # fused: [kt|qt] = (S*[k|q] * 1/S) * [invp|cp]
                    nc.vector.scalar_tensor_tensor(
                        ktqt[:], kq[:], 1.0 / S, w[:], ALU.mult, ALU.mult)
                else:
                    kqs = work.tile([N, 512], F32, tag="kqs", name="kqs")
                    nc.vector.tensor_scalar(kqs[:], kq[:], 1.0 / S, 0.0,
                                            ALU.mult, ALU.add)
                    nc.vector.scalar_tensor_tensor(
                        kt, kqs[:, 0:256], bias_sb["bks"][:], w[:, 0:256],
                        ALU.add, ALU.mult)
                    nc.vector.scalar_tensor_tensor(
                        qt, kqs[:, 256:512], bias_sb["bqs"][:], w[:, 256:512],
                        ALU.add, ALU.mult)

                if p + 1 < NPAIR:
                    head[p + 1] = emit_gate_head(p + 1)

            def emit_y_block():
                ys = yout.tile([C, 2, D], F16, tag="ys", name="ys")
                if not last:
                    for h in range(2):
                        yp = ps_y.tile([C, D], F32, tag="y", name=f"yp{h}")
                        nc.tensor.matmul(yp[:], atm[:, h * C : (h + 1) * C],
                                         v_p[h][:], start=True, stop=True)
                        # h0 evac on ACT, h1 on DVE (engine balance)
                        if h == 0:
                            nc.scalar.copy(ys[:, 0, :], yp[:])
                        else:
                            nc.vector.tensor_copy(ys[:, 1, :], yp[:])
                    nc.sync.dma_start(
                        y[cpair * 2 * C : (cpair + 1) * 2 * C, :]
                        .rearrange("(h p) d -> p h d", p=C),
                        ys[:],
                    )
                    return
                # Final pair: chunk h0 full-width; chunk h1 as two half-d
                # matmuls with parallel ACT/DVE evacs and a single DMA, both
                # DMAs on the idle sync queue (lower DGE delay).
                yp0 = ps_y.tile([C, D], F32, tag="y", name="yp0")
                nc.tensor.matmul(yp0[:], atm[:, 0:C], v_p[0][:],
                                 start=True, stop=True)
                nc.vector.tensor_copy(ys[:, 0, :], yp0[:])
                c0 = cpair * 2
                nc.sync.dma_start(y[c0 * C : (c0 + 1) * C, :], ys[:, 0, :])
                yp1 = ps_y.tile([C, D], F32, tag="y", name="yp1")
                nc.tensor.matmul(yp1[:], atm[:, C : 2 * C], v_p[1][:],
                                 start=True, stop=True)
                nc.scalar.copy(ys[:, 1, 0:256], yp1[:, 0:256])
                nc.vector.tensor_copy(ys[:, 1, 256:512], yp1[:, 256:512])
                c1 = cpair * 2 + 1
                nc.sync.dma_start(y[c1 * C : (c1 + 1) * C, :], ys[:, 1, :])

            def emit_v_chunk(cc):
                    vp = ps_v.tile([C, D], F32, tag="v", name="v")
                    mms = []
                    for h in range(2):
                        mms.append((x8_st(p, h, cc), wv[:, 0, 2 * h : 2 * h + 2, :]))
                    for h in range(2):
                        mms.append((x8_st(p, h, cc), wv[:, 2, 2 * h : 2 * h + 2, :]))
                    for h in range(2):
                        mms.append((r8_st(p, h, cc), wv[:, 1, 2 * h : 2 * h + 2, :]))
                    for j, (lhsT, rhs) in enumerate(mms):
                        nc.tensor.matmul(vp[:], lhsT, rhs,
                                         start=(j == 0), stop=(j == 5),
                                         perf_mode=PM.DoubleRow)
                    vs = vout.tile([C, D], BF16, tag="vs", name="vs")
                    if bias_sb is None:
                        nc.scalar.activation(vs[:], vp[:], AF.Copy,
                                             scale=1.0 / S)
                    else:
                        nc.vector.scalar_tensor_tensor(
                            vs[:], vp[:], 1.0 / S, bias_sb["bv_full"][:],
                            ALU.mult, ALU.add)
                    return vs

            def emit_v_block():
                return [emit_v_chunk(0), emit_v_chunk(1)]

            if stage_c:
                emit_y_block()
            if stage_a:
                if p == NPAIR - 1:
                    v_sb = [emit_v_chunk(0)]
                    atm_pre = emit_at_mask(kt, qt)
                    v_sb.append(emit_v_chunk(1))
                else:
                    atm_pre = emit_at_mask(kt, qt)
                    v_sb = emit_v_block()
                hist[p] = (v_sb, atm_pre)


_NC_CACHE = {}


def _get_nc(zero_bias=True):
    if zero_bias not in _NC_CACHE:
        _NC_CACHE[zero_bias] = build_nc(zero_bias)
    return _NC_CACHE[zero_bias]


def _q8(a):
    return np.asarray(a, np.float32).astype(NP_F8)


def _pack_w(w, cols):
    """(cols, I) f32 -> [128, 4, cols] with i = 128j + p (fp32 values)."""
    wT = np.asarray(w, np.float32).T.reshape(4, 128, cols)
    return np.ascontiguousarray(wT.transpose(1, 0, 2))


def _w_terms(w, cols):
    """3-term fp8 set for one weight matrix: (W1, W2, R1), each [128,4,cols]."""
    wp = _pack_w(w, cols)
    w1 = _q8(S * wp)
    w2 = _q8((S / RS) * wp)
    r1 = _q8(S * wp - w1.astype(np.float32))
    return w1, w2, r1


def make_in_maps(x, Wv, bv, Wk, bk, Wq, bq, Wa, ba, zero_bias=True):
    x = np.asarray(x, np.float32)

    kv1, kv2, kr1 = _w_terms(Wk, N)
    qv1, qv2, qr1 = _w_terms(Wq, N)
    av1, _, _ = _w_terms(Wa, N)
    wkqa = np.ascontiguousarray(
        np.stack([kv1, kv2, kr1, qv1, qv2, qr1, av1], axis=1))

    vv1, vv2, vr1 = _w_terms(Wv, D)
    wv = np.ascontiguousarray(np.stack([vv1, vv2, vr1], axis=1))

    shared = {"wkqa_r": wkqa, "wv_r": wv}
    if not zero_bias:
        shared.update({
            "bv": np.asarray(bv, np.float32).reshape(1, D),
            "bks": np.asarray(bk, np.float32).reshape(N, 1),
            "bqs": np.asarray(bq, np.float32).reshape(N, 1),
            "ba": np.asarray(ba, np.float32).reshape(N, 1),
        })

    in_maps = []
    for b in range(NCORES):
        xb = x[:, b, :]                       # (T, I)
        x8 = _q8(xb)
        r8 = _q8(RS * (xb - x8.astype(np.float32)))
        # [p, pair, j, t]: val[t, 128j+p] with t = 256*pair + tau
        def pk(a):
            return a.T.reshape(4, 128, NPAIR, 256).transpose(1, 2, 0, 3)
        xr = np.concatenate([pk(x8), pk(r8)], axis=2)  # [128, NPAIR, 8, 256]
        in_maps.append({"xr8_r": np.ascontiguousarray(xr), **shared})
    return in_maps


def run(inputs, trace=False, **kw):
    zero_bias = all(
        not np.any(np.asarray(inputs[k])) for k in ("bv", "bk", "bq", "ba")
    )
    nc = _get_nc(zero_bias)
    in_maps = make_in_maps(**inputs, zero_bias=zero_bias)
    res = run_bass_kernel_spmd(nc, in_maps, core_ids=list(range(NCORES)),
                               trace=trace, **kw)
    out = np.stack(
        [res.results[b]["y"].astype(np.float32) for b in range(NCORES)], axis=1
    )
    return out, res


def kernel(x, Wv, bv, Wk, bk, Wq, bq, Wa, ba):
    out, _ = run(dict(x=x, Wv=Wv, bv=bv, Wk=Wk, bk=bk, Wq=Wq, bq=bq,
                      Wa=Wa, ba=ba))
    return out
